# revision 5
# baseline (speedup 1.0000x reference)
"""Trainium2 Bass kernel for gnn_message_passing saliency (2x3x128x128).

Math: reference = per-pixel cosine-similarity saliency.
  bu(p) = LayerNorm_per_channel(unfold7(image))(p) + 2            [B,T,147]
  cos(p,q) for q in 5x5\{p};  sal = mean_valid((1-cos)/2)

Because LayerNorm subtracts the patch mean, cross terms vanish and
  dot(bu_p, bu_q) = sum_c [(C_c(p,d) - 49 mu_c(p) mu_c(q)) r_c(p) r_c(q)] + 588
  ||bu_p||^2      = sum_c [49 var_c(p) r_c(p)^2] + 588
with C_c(p,d) = 7x7 box sum of A_c * shift(A_c, d), r = 1/sqrt(var+eps).
So the whole kernel reduces to per-channel box filters of A, A^2 and 12
shifted products (the other 12 offsets follow by symmetry
cos(p,-d) = cos(p-d,d)).

Sharding: 8 cores = 2 images x 4 row-slabs of 32 rows (SPMD-uniform; all
per-core differences are input data).  On-device layout: partitions = W
(128), free dim = rows.  Box filters: rows via DVE prefix scan + shifted
difference folded into the PE band matmul (+B7 / -B7 accumulated in PSUM),
W via the same PE band matmul over partitions.
"""

import functools
import os
import sys
from contextlib import ExitStack

import numpy as np

sys.path.insert(0, "/opt/trn_rl_repo")

B, C, H, W = 2, 3, 128, 128
KC, KN = 7, 5
EPS_LN = 1e-5
NCORES = 8
SLABS = 4
RS = H // SLABS  # 32 rows per slab

# 12 representative offsets (di=row shift, dj=col shift); the other 12 are -d.
D12 = [(0, 1), (0, 2)] + [(1, dj) for dj in (-2, -1, 0, 1, 2)] + [
    (2, dj) for dj in (-2, -1, 0, 1, 2)
]
DJS = (-2, -1, 0, 1, 2)  # variant order for partition(W)-shifted copies
# op groups: (d_start, n_d, di, v_start) with dj consecutive inside a group
GROUPS = [(0, 2, 0, 3), (2, 5, 1, 0), (7, 5, 2, 0)]

AT = 43  # `a` slot rows   [r0-6, r0+36]
PT = 41  # product slot    [r0-6, r0+34]
ST = 36  # stats rows      [r0-2, r0+33]
YT = 34  # dsum/E rows     [r0-2, r0+31]
OT = 32  # output rows     [r0,   r0+31]


def _host_constants():
    k = np.arange(W)
    b7 = (np.abs(k[:, None] - k[None, :]) <= 3).astype(np.float32)
    bpm = np.concatenate([b7, -b7], axis=1)  # [128, 256]
    sh = np.zeros((W, 5 * W), np.float32)
    for v, dj in enumerate(DJS):
        m = np.arange(W)
        kk = m - dj
        ok = (kk >= 0) & (kk < W)
        sh[kk[ok], v * W + m[ok]] = 1.0
    # count of valid neighbours among the 24 offsets, per pixel
    ii = np.arange(H)[:, None]
    jj = np.arange(W)[None, :]
    cnt = np.zeros((H, W), np.float32)
    for di in range(-2, 3):
        for dj in range(-2, 3):
            if di == 0 and dj == 0:
                continue
            cnt += (
                (ii + di >= 0) & (ii + di < H) & (jj + dj >= 0) & (jj + dj < W)
            ).astype(np.float32)
    return bpm, sh, cnt


def _core_inputs(image, bpm, sh, cnt):
    """Per-core input dicts. image: [B,C,H,W] float32."""
    maps = []
    for core in range(NCORES):
        img, slab = divmod(core, SLABS)
        r0 = slab * RS
        # a rows r0-6 .. r0+36 zero-padded, W-major: a_rows[c, t, w]
        a_rows = np.zeros((C, AT, W), np.float32)
        lo, hi = r0 - 6, r0 + 37
        clo, chi = max(lo, 0), min(hi, H)
        a_rows[:, clo - lo : chi - lo, :] = image[img, :, clo:chi, :]
        # A5[w, v, c, t] = a_rows[c, t, w + DJS[v]]
        a_w = a_rows.transpose(2, 0, 1)  # [w, c, t]
        a5 = np.zeros((W, 5, C, AT), np.float32)
        for v, dj in enumerate(DJS):
            wlo, whi = max(0, -dj), min(W, W - dj)
            a5[wlo:whi, v] = a_w[wlo + dj : whi + dj]
        msk = np.zeros((W, ST), np.float32)
        rows = r0 - 2 + np.arange(ST)
        msk[:, (rows >= 0) & (rows < H)] = 1.0
        rcp = (-0.5 / cnt[r0 : r0 + RS, :]).T.copy()  # [w, 32]
        maps.append(
            {
                "a5": a5.reshape(W, 5 * C * AT),
                "bpm": bpm,
                "sh": sh,
                "mask": msk,
                "rcp": np.ascontiguousarray(rcp),
            }
        )
    return maps


@functools.lru_cache(maxsize=1)
def _build_nc():
    import concourse.bacc as bacc
    import concourse.tile as tile
    from concourse import mybir

    f32 = mybir.dt.float32
    add = mybir.AluOpType.add
    mult = mybir.AluOpType.mult
    bypass = mybir.AluOpType.bypass
    X = mybir.AxisListType.X
    Sqrt = mybir.ActivationFunctionType.Sqrt
    Ident = mybir.ActivationFunctionType.Identity

    nc = bacc.Bacc("TRN2", target_bir_lowering=False, debug=False,
                   num_devices=NCORES)
    # const APs for activation biases (same pattern as the built-in 0.0/1.0)
    for val in (EPS_LN, 588.0, 0.5):
        t = nc.alloc_sbuf_tensor(f"const-f32-{val}", [128, 1], f32)
        nc.gpsimd.memset(t.ap(), val)
        nc.const_aps.aps[(f32, val)] = t.ap()
    nc.all_engine_barrier()
    a5d = nc.dram_tensor("a5", [W, 5 * C * AT], f32, kind="ExternalInput").ap()
    bpmd = nc.dram_tensor("bpm", [W, 256], f32, kind="ExternalInput").ap()
    shd = nc.dram_tensor("sh", [W, 5 * W], f32, kind="ExternalInput").ap()
    mkd = nc.dram_tensor("mask", [W, ST], f32, kind="ExternalInput").ap()
    rcd = nc.dram_tensor("rcp", [W, OT], f32, kind="ExternalInput").ap()
    outd = nc.dram_tensor("out", [W, OT], f32, kind="ExternalOutput").ap()

    with tile.TileContext(nc) as tc:
        with ExitStack() as ctx:
            sb = ctx.enter_context(tc.tile_pool(name="sb", bufs=1))
            ps = ctx.enter_context(tc.tile_pool(name="ps", bufs=1, space="PSUM"))

            A5 = sb.tile([W, 5 * C * AT], f32)
            for v in range(5):
                nc.sync.dma_start(
                    A5[:, v * C * AT : (v + 1) * C * AT],
                    a5d[:, v * C * AT : (v + 1) * C * AT],
                )
            BPM = sb.tile([W, 256], f32)
            nc.sync.dma_start(BPM[:], bpmd)
            SH = sb.tile([W, 5 * W], f32)
            nc.sync.dma_start(SH[:], shd)
            MK = sb.tile([W, ST], f32)
            nc.sync.dma_start(MK[:], mkd)
            RC = sb.tile([W, OT], f32)
            nc.sync.dma_start(RC[:], rcd)

            A5v = A5[:].rearrange("p (v c t) -> p v c t", v=5, c=C)
            a_c = A5[:, 2 * C * AT : 3 * C * AT]  # unshifted a, [128, 129]

            # ---- stats: S1/S2 7x7 box of a and a^2 --------------------
            A2 = sb.tile([W, C * AT], f32)
            nc.scalar.square(A2[:], a_c)
            Sa = sb.tile([W, C * AT], f32)
            nc.vector.tensor_tensor_scan(Sa[:], a_c, a_c, 0.0, add, bypass)
            Sa2 = sb.tile([W, C * AT], f32)
            nc.vector.tensor_tensor_scan(Sa2[:], A2[:], A2[:], 0.0, add, bypass)

            SBp = ps.tile([W, 216], f32)
            Sav = Sa[:].rearrange("p (c t) -> p c t", c=C)
            Sa2v = Sa2[:].rearrange("p (c t) -> p c t", c=C)
            for off, src in ((0, Sav), (108, Sa2v)):
                nc.tensor.matmul(SBp[:, off : off + 108], BPM[:, 0:128],
                                 src[:, :, 7 : 7 + ST], start=True, stop=False)
                nc.tensor.matmul(SBp[:, off : off + 108], BPM[:, 128:256],
                                 src[:, :, 0:ST], start=False, stop=True)

            m = sb.tile([W, C * ST], f32)
            nc.scalar.mul(m[:], SBp[:, 0:108], 1.0 / 7.0)
            q = sb.tile([W, C * ST], f32)
            nc.vector.tensor_mul(q[:], m[:], m[:])
            wv = sb.tile([W, C * ST], f32)
            nc.vector.tensor_sub(wv[:], SBp[:, 108:216], q[:])
            sg = sb.tile([W, C * ST], f32)
            nc.scalar.activation(sg[:], wv[:], Sqrt, bias=EPS_LN, scale=1.0 / 49.0)

            R5 = sb.tile([W, 5 * C * ST], f32)
            scr = sb.tile([W, C * ST], f32)
            r_sl = R5[:, 2 * C * ST : 3 * C * ST]
            nc.gpsimd.memset(R5[:], 0.0)
            nc.vector.reciprocal_approx_accurate(out=r_sl, in_=sg[:], scratch=scr[:])
            G5 = sb.tile([W, 5 * C * ST], f32)
            g_sl = G5[:, 2 * C * ST : 3 * C * ST]
            nc.gpsimd.memset(G5[:], 0.0)
            nc.vector.tensor_mul(g_sl, m[:], r_sl)

            e1 = sb.tile([W, C * ST], f32)
            nc.vector.tensor_mul(e1[:], wv[:], r_sl)
            e2 = sb.tile([W, C * ST], f32)
            nc.vector.tensor_mul(e2[:], e1[:], r_sl)
            n2 = sb.tile([W, ST], f32)
            nc.vector.tensor_reduce(
                n2[:], e2[:].rearrange("p (c s) -> p s c", c=C), X, add
            )
            nn = sb.tile([W, ST], f32)
            nc.scalar.activation(nn[:], n2[:], Sqrt, bias=588.0, scale=1.0)
            rn0 = sb.tile([W, ST], f32)
            scr2 = sb.tile([W, ST], f32)
            nc.vector.reciprocal_approx_accurate(out=rn0[:], in_=nn[:], scratch=scr2[:])
            RN5 = sb.tile([W, 5 * ST], f32)
            nc.gpsimd.memset(RN5[:], 0.0)
            nc.vector.tensor_mul(RN5[:, 2 * ST : 3 * ST], rn0[:], MK[:])

            # partition(W)-shifted copies of r, g, rn (tiles pre-zeroed, so
            # the partitions the shifted copy can't fill stay 0)
            for v, dj in ((0, -2), (1, -1), (3, 1), (4, 2)):
                for t, wd in ((R5, C * ST), (G5, C * ST), (RN5, ST)):
                    dst = t[:, v * wd : (v + 1) * wd]
                    src = t[:, 2 * wd : 3 * wd]
                    if dj > 0:
                        nc.sync.dma_start(dst[0 : W - dj, :], src[dj:W, :])
                    else:
                        nc.sync.dma_start(dst[-dj:W, :], src[0 : W + dj, :])

            # ---- products P_d = a * shift(a, d) ------------------------
            Pall = sb.tile([W, 12 * C * PT], f32)
            Pv = Pall[:].rearrange("p (d c x) -> p d c x", d=12, c=C)
            for gi, (d0, nd, di, v0) in enumerate(GROUPS):
                in0 = A5v[:, 2:3, :, 0:PT].broadcast_to((W, nd, C, PT))
                in1 = A5v[:, v0 : v0 + nd, :, di : di + PT]
                eng = nc.vector if gi == 0 else nc.gpsimd
                eng.tensor_mul(Pv[:, d0 : d0 + nd], in0, in1)

            SP = sb.tile([W, 12 * C * PT], f32)
            nc.vector.tensor_tensor_scan(SP[:], Pall[:], Pall[:], 0.0, add, bypass)
            SPv = SP[:].rearrange("p (d c x) -> p d c x", d=12, c=C)

            # ---- C = 7x7 box of P via +-band matmuls -------------------
            Cp = ps.tile([W, 1536], f32)
            for b in range(3):
                o = 512 * b
                nc.tensor.matmul(Cp[:, o : o + 408], BPM[:, 0:128],
                                 SPv[:, 4 * b : 4 * b + 4, :, 7 : 7 + YT],
                                 start=True, stop=False)
                nc.tensor.matmul(Cp[:, o : o + 408], BPM[:, 128:256],
                                 SPv[:, 4 * b : 4 * b + 4, :, 0:YT],
                                 start=False, stop=True)

            # ---- rr, gg, u, t, dsum ------------------------------------
            R5v = R5[:].rearrange("p (v c s) -> p v c s", v=5, c=C)
            G5v = G5[:].rearrange("p (v c s) -> p v c s", v=5, c=C)
            RR = sb.tile([W, 12 * C * YT], f32)
            RRv = RR[:].rearrange("p (d c y) -> p d c y", d=12, c=C)
            GG = sb.tile([W, 12 * C * YT], f32)
            GGv = GG[:].rearrange("p (d c y) -> p d c y", d=12, c=C)
            for d0, nd, di, v0 in GROUPS:
                in0 = R5v[:, 2:3, :, 0:YT].broadcast_to((W, nd, C, YT))
                in1 = R5v[:, v0 : v0 + nd, :, di : di + YT]
                nc.vector.tensor_mul(RRv[:, d0 : d0 + nd], in0, in1)
                in0g = G5v[:, 2:3, :, 0:YT].broadcast_to((W, nd, C, YT))
                in1g = G5v[:, v0 : v0 + nd, :, di : di + YT]
                nc.gpsimd.tensor_mul(GGv[:, d0 : d0 + nd], in0g, in1g)

            U = sb.tile([W, 12 * C * YT], f32)
            Cv = Cp[:].rearrange("p (b n) -> p b n", b=3)[:, :, 0:408]
            nc.vector.tensor_mul(
                U[:].rearrange("p (b n) -> p b n", b=3), Cv,
                RR[:].rearrange("p (b n) -> p b n", b=3),
            )
            T = sb.tile([W, 12 * C * YT], f32)
            nc.gpsimd.tensor_sub(T[:], U[:], GG[:])
            dsum = sb.tile([W, 12 * YT], f32)
            nc.vector.tensor_reduce(
                dsum[:], T[:].rearrange("p (d c y) -> p d y c", d=12, c=C), X, add
            )

            # ---- cos maps: E (for -d side), D (for +d side) ------------
            dsv = dsum[:].rearrange("p (d y) -> p d y", d=12)
            RN5v = RN5[:].rearrange("p (v s) -> p v s", v=5)
            E = sb.tile([W, 12 * YT], f32)
            Ev = E[:].rearrange("p (d y) -> p d y", d=12)
            nc.vector.scalar_tensor_tensor(
                Ev, dsv, 588.0, RN5v[:, 2:3, 0:YT].broadcast_to((W, 12, YT)),
                add, mult,
            )
            D = sb.tile([W, 12 * OT], f32)
            Dv = D[:].rearrange("p (d y) -> p d y", d=12)
            for d0, nd, di, v0 in GROUPS:
                nc.vector.scalar_tensor_tensor(
                    Dv[:, d0 : d0 + nd], dsv[:, d0 : d0 + nd, 2 : 2 + OT], 588.0,
                    RN5v[:, v0 : v0 + nd, 2 + di : 2 + di + OT], add, mult,
                )
            Dsum = sb.tile([W, OT], f32)
            nc.vector.tensor_reduce(
                Dsum[:], D[:].rearrange("p (d y) -> p y d", d=12), X, add
            )
            Ep = ps.tile([W, OT], f32)
            SHv = SH[:].rearrange("p (v m) -> p v m", v=5)
            for i, (di, dj) in enumerate(D12):
                nc.tensor.matmul(Ep[:], SHv[:, dj + 2],
                                 Ev[:, i, 2 - di : 2 - di + OT],
                                 start=(i == 0), stop=(i == 11))

            acc = sb.tile([W, OT], f32)
            nc.vector.tensor_add(acc[:], Dsum[:], Ep[:])
            acc2 = sb.tile([W, OT], f32)
            nc.vector.tensor_mul(acc2[:], acc[:], RN5v[:, 2, 2 : 2 + OT])
            t4 = sb.tile([W, OT], f32)
            nc.vector.tensor_mul(t4[:], acc2[:], RC[:])
            outT = sb.tile([W, OT], f32)
            nc.scalar.activation(outT[:], t4[:], Ident, bias=0.5, scale=1.0)
            nc.sync.dma_start(outd, outT[:])

    nc.compile()
    return nc


def kernel(image: np.ndarray) -> np.ndarray:
    from concourse.bass_utils import run_bass_kernel_spmd

    image = np.asarray(image, dtype=np.float32)
    bpm, sh, cnt = _host_constants()
    in_maps = _core_inputs(image, bpm, sh, cnt)
    nc = _build_nc()
    res = run_bass_kernel_spmd(nc, in_maps, list(range(NCORES)))
    sal = np.zeros((B, H, W), np.float32)
    for core in range(NCORES):
        img, slab = divmod(core, SLABS)
        r0 = slab * RS
        sal[img, r0 : r0 + RS, :] = res.results[core]["out"].T
    return sal
